# revision 11
# baseline (speedup 1.0000x reference)
"""Weighted BCE loss (nn_BCELoss_with_weight) on 8 Trainium2 NeuronCores.

Reference computes:
    log_p   = max(log(pred), -100)            # clamp never binds: pred in [1e-4, 1-1e-4]
    log_1mp = max(log1p(-pred), -100)
    bce     = -(true*log_p + (1-true)*log_1mp)    # [B,C,D,H,W] = [2,16,64,128,128]
    per_class = mean(bce, axes=(0,2,3,4))         # [C]
    out = sum(weight*per_class) / sum(weight)     # scalar

Sharding: D=64 split into 8 slices of 8 (data parallel). Per core the shard
[2,16,8,128,128] is viewed as [B=2, (C,Dl)=128, H*W=16384]: partition p holds
class c=p//8 only, so the per-class weight is a per-partition scalar.

Per core on device, with u=ln(p), v=ln(1-p), w~=bf16(weight):
    term = t*u + (1-t)*v = t*(u-v) + v
    DMA : pred f32 on the sync HWDGE ring (sequencer issues nothing else, so
          issue never blocks behind compute); true on gpsimd SWDGE with inline
          f32->bf16 cast.
    ACT : u = Ln(p) [bf16], v = Ln(-p+1) [bf16]
    DVE : d = u - v (bf16 TT 2x);  m = t*d (bf16 TT 2x)
    PE  : psum[1,512] += wf[128,1].T @ v_chunk  and  += wf.T @ m_chunk
          (both streams weighted by wf and accumulated in one f32 PSUM bank)
    out[1,1] = sum(psum)   -- single 4-byte output, one DMA descriptor
          (a [128,1] output would be 128 4-byte HBM read-modify-writes whose
          completion receipts serialize ~6us on the SDMA engines)
Host: result = -(sum_cores out) / (M * sum(w~)), M = B*D*H*W. Using the
bf16-rounded weights consistently in numerator and denominator makes this the
exact weighted mean of per-class BCE with weights w~; per-class means are
~equal so the w->w~ rounding perturbs the result by ~1e-5 relative.
"""

import numpy as np

N_CORES = 8
B, C, D, H, W = 2, 16, 64, 128, 128
HW = H * W            # 16384 free elems per (b, partition)
P = 128               # (C=16) x (D_local=8) partitions
D_LOCAL = D // N_CORES
MM_N = 512            # one PSUM bank of f32

# Per-b DMA segment plans: mids big for DMA/ACT efficiency, small tail so the
# last chunk's LN->DVE->PE chain after the final byte is short.
SEGS_B0 = (1024, 2048, 2048, 2048, 2048, 2048, 2048, 2048, 1024)
SEGS_B1 = (2048, 2048, 2048, 2048, 2048, 2048, 2048, 1024, 512, 512)


def build_bass_kernel(segs_b0=SEGS_B0, segs_b1=SEGS_B1,
                      pin_bufs=10, tin_bufs=10, uv_bufs=5, m_bufs=4,
                      sub=2048, mul_lag=2, alternate=False,
                      direct_reduce=True):
    """Build the per-core Bass/Tile kernel.

    Inputs  : pred, true [B, 128, free] f32 (shard, class*d_local on axis 1)
              wf [128, 1] bf16 (per-partition class weight)
    Outputs : out_m [1, 1] f32 = sum_p wf[p] * sum_e (t*(u-v) + v)[p, e]
    """
    import concourse.bacc as bacc
    import concourse.mybir as mybir
    import concourse.tile as tile

    f32 = mybir.dt.float32
    bf16 = mybir.dt.bfloat16
    AF = mybir.ActivationFunctionType

    segs_per_b = [list(segs_b0), list(segs_b1)]
    for segs in segs_per_b:
        assert sum(segs) == HW, segs
    plan = []                       # (b, offset, seg)
    total_mm = 0
    for b in range(B):
        off = 0
        for seg in segs_per_b[b]:
            plan.append((b, off, seg))
            total_mm += 2 * max(1, seg // MM_N)
            off += seg

    nc = bacc.Bacc("TRN2", target_bir_lowering=False, debug=False,
                   num_devices=N_CORES)
    pred_d = nc.dram_tensor("pred", [B, P, HW], f32, kind="ExternalInput")
    true_d = nc.dram_tensor("true", [B, P, HW], f32, kind="ExternalInput")
    wf_d = nc.dram_tensor("wf", [P, 1], bf16, kind="ExternalInput")
    outm_d = nc.dram_tensor("out_m", [1, 1], f32, kind="ExternalOutput")

    with tile.TileContext(nc) as tc:
        n_head = sum(1 for s in segs_b0 if s > sub)
        with (
            tc.tile_pool(name="headp", bufs=max(n_head, 1)) as headp,
            tc.tile_pool(name="pin", bufs=pin_bufs) as pin,
            tc.tile_pool(name="tin", bufs=tin_bufs) as tin,
            tc.tile_pool(name="uv", bufs=uv_bufs) as uvp,
            tc.tile_pool(name="mp", bufs=m_bufs) as mp,
            tc.tile_pool(name="small", bufs=1) as small,
            tc.tile_pool(name="psum", bufs=1, space="PSUM") as psump,
        ):
            wf_t = small.tile([P, 1], bf16, tag="wf")
            nc.gpsimd.dma_start(wf_t[:], wf_d[:])
            acc = psump.tile([1, MM_N], f32, tag="acc")
            # warm up the Ln table set so the first real ACTIVATE doesn't pay
            # the ~2.7us ACT_TABLE_LOAD after its data lands. Input comes from
            # a memset (not the wf DMA) so the warm-up never blocks the ACT
            # FIFO behind a DMA-completion semaphore.
            warm_in = small.tile([P, 1], f32, tag="warm_in")
            nc.vector.memset(warm_in[:], 1.0)
            warm = small.tile([P, 1], bf16, tag="warm")
            nc.scalar.activation(warm[:], warm_in[:], AF.Ln, bias=1.0,
                                 scale=1.0)

            mm_i = 0
            # Pipeline the t-dependent DVE muls `mul_lag` sub-chunks behind
            # the subs: a mul waiting on its true-chunk DMA must not
            # head-of-line-block the next sub in DVE's FIFO (that stall
            # cascades: uv recycling -> ACT -> pin recycling -> pred DMA).
            pending = []        # (m_tile, t_tile, t_slice, width)

            def mm(src, w):
                nonlocal mm_i
                for q in range(max(1, w // MM_N)):
                    qq = slice(q * MM_N, min((q + 1) * MM_N, w))
                    nc.tensor.matmul(acc[:, 0:qq.stop - qq.start],
                                     wf_t[:], src[:, qq],
                                     start=(mm_i == 0),
                                     stop=(mm_i == total_mm - 1))
                    mm_i += 1

            def flush_one():
                m_t, t_t, tss, w = pending.pop(0)
                nc.vector.tensor_mul(m_t[:], t_t[:, tss], m_t[:])
                mm(m_t, w)

            for pi, (b, off, seg) in enumerate(plan):
                # ramp segs larger than `sub` live in their own pool so they
                # don't inflate every recycled pin slot
                p_pool = headp if (pi < n_head and seg > sub) else pin
                p_t = p_pool.tile([P, seg], f32, tag="p")
                t_t = tin.tile([P, seg], bf16, tag="t")
                sl = slice(off, off + seg)
                nc.sync.dma_start(p_t[:], pred_d[b, :, sl])
                # f32 -> bf16 cast inline (SWDGE-only feature)
                nc.gpsimd.dma_start(t_t[:], true_d[b, :, sl])
                s_off = 0
                while s_off < seg:
                    s_sz = min(sub, seg - s_off)
                    ss = slice(s_off, s_off + s_sz)
                    u = uvp.tile([P, s_sz], bf16, tag="u")
                    v = uvp.tile([P, s_sz], bf16, tag="v")
                    nc.scalar.activation(u[:], p_t[:, ss], AF.Ln,
                                         bias=0.0, scale=1.0)
                    nc.scalar.activation(v[:], p_t[:, ss], AF.Ln,
                                         bias=1.0, scale=-1.0)
                    # acc += wf.T @ v (v is ready first; PE runs these while
                    # DVE forms m), later acc += wf.T @ m
                    mm(v, s_sz)
                    # d = u - v into a separate tile so u/v recycle without
                    # waiting on the t-gated mul
                    m_t = mp.tile([P, s_sz], bf16, tag="m")
                    nc.vector.tensor_sub(m_t[:], u[:], v[:])
                    pending.append((m_t, t_t, ss, s_sz))
                    while len(pending) > mul_lag:
                        flush_one()
                    s_off += s_sz
            while pending:
                flush_one()
            assert mm_i == total_mm

            outm_t = small.tile([1, 1], f32, tag="outm")
            if direct_reduce:
                nc.vector.reduce_sum(outm_t[:], acc[:],
                                     axis=mybir.AxisListType.X)
            else:
                accm_sb = small.tile([1, MM_N], f32, tag="accm_sb")
                nc.vector.tensor_copy(accm_sb[:], acc[:])
                nc.vector.reduce_sum(outm_t[:], accm_sb[:],
                                     axis=mybir.AxisListType.X)
            nc.sync.dma_start(outm_d[:], outm_t[:])

    nc.compile()
    return nc


_NC_CACHE = {}


def _get_nc():
    if "nc" not in _NC_CACHE:
        import json
        import os

        opts = json.loads(os.environ.get("KERNEL_OPTS", "{}"))
        for k in ("segs_b0", "segs_b1"):
            if k in opts:
                opts[k] = tuple(opts[k])
        _NC_CACHE["nc"] = build_bass_kernel(**opts)
    return _NC_CACHE["nc"]


def _bf16_round(x):
    """Round f32 array to bf16 values (kept in f32 representation)."""
    xi = np.asarray(x, dtype=np.float32).view(np.uint32)
    rounded = ((xi + 0x7FFF + ((xi >> 16) & 1)) & 0xFFFF0000).astype(np.uint32)
    return rounded.view(np.float32)


def shard_inputs(pred, true, weight):
    """Full [B,C,D,H,W] -> per-core in_maps."""
    import ml_dtypes

    wtile = np.repeat(np.asarray(weight, np.float32), D_LOCAL).reshape(P, 1)
    wf = wtile.astype(ml_dtypes.bfloat16)
    in_maps = []
    for i in range(N_CORES):
        d0 = i * D_LOCAL
        ps = np.ascontiguousarray(
            pred[:, :, d0:d0 + D_LOCAL].reshape(B, P, HW))
        ts = np.ascontiguousarray(
            true[:, :, d0:d0 + D_LOCAL].reshape(B, P, HW))
        in_maps.append({"pred": ps, "true": ts, "wf": wf})
    return in_maps


def combine(out_ms, weight):
    """out_ms [n_cores] scalars; weight [16] f32."""
    wt = _bf16_round(np.repeat(np.asarray(weight, np.float32), D_LOCAL))
    m = float(B * D * H * W)
    w_sum = wt.astype(np.float64)[::D_LOCAL].sum()   # sum of bf16 class weights
    total = float(np.asarray(out_ms, np.float64).sum())
    return np.float32(-total / (m * w_sum))


def kernel(pred, true, weight, _trace=False):
    from concourse.bass_utils import run_bass_kernel_spmd

    nc = _get_nc()
    in_maps = shard_inputs(np.asarray(pred), np.asarray(true), weight)
    res = run_bass_kernel_spmd(nc, in_maps, core_ids=list(range(N_CORES)),
                               trace=_trace)
    out_ms = [r["out_m"][0, 0] for r in res.results]
    out = combine(out_ms, weight)
    if _trace:
        return out, res
    return out


# revision 17
# speedup vs baseline: 1.0847x; 1.0847x over previous
"""Weighted BCE loss (nn_BCELoss_with_weight) on 8 Trainium2 NeuronCores.

Reference computes:
    log_p   = max(log(pred), -100)            # clamp never binds: pred in [1e-4, 1-1e-4]
    log_1mp = max(log1p(-pred), -100)
    bce     = -(true*log_p + (1-true)*log_1mp)    # [B,C,D,H,W] = [2,16,64,128,128]
    per_class = mean(bce, axes=(0,2,3,4))         # [C]
    out = sum(weight*per_class) / sum(weight)     # scalar

Sharding: D=64 split into 8 slices of 8 (data parallel). Per core the shard
[2,16,8,128,128] is viewed as [B=2, (C,Dl)=128, H*W=16384]: partition p holds
class c=p//8 only, so the per-class weight is a per-partition scalar.

Per core on device, with u=ln(p), v=ln(1-p), w~=bf16(weight):
    term = t*u + (1-t)*v = t*(u-v) + v
    DMA : pred f32 on the sync HWDGE ring (that sequencer issues nothing
          else, so issue never blocks behind compute; pin_bufs=14 covers
          nearly the whole stream so the tail is never issue-gated by
          recycling); true on gpsimd SWDGE with inline f32->bf16 cast,
          deliberately shallow (tin_bufs=3) so its issue is paced by mul
          progress -- the t stream then cannot out-compete pred for SDMA
          service (the two queues round-robin at packet granularity and
          pred must stay ahead: it feeds both Ln passes).
    ACT : u = Ln(p) [bf16], v = Ln(-p+1) [bf16]
    DVE : d = u - v into a separate m-tile (so u/v recycle without waiting
          on t), then m = t*d, software-pipelined one chunk behind so a mul
          waiting on its t-chunk DMA never head-of-line-blocks the next sub.
    PE  : psum[1,512] += wf[128,1].T @ v_chunk  and  += wf.T @ m_chunk
          (both streams weighted by wf and accumulated in one f32 PSUM bank)
    out[1,1] = sum(psum)   -- single 4-byte output, one DMA descriptor
          (a [128,1] output would be 128 4-byte HBM read-modify-writes whose
          completion receipts serialize ~6us on the SDMA engines)
Host: result = -(sum_cores out) / (M * sum(w~)), M = B*D*H*W. Using the
bf16-rounded weights consistently in numerator and denominator makes this the
exact weighted mean of per-class BCE with weights w~; per-class means are
~equal so the w->w~ rounding perturbs the result by ~1e-5 relative.

Measured on 8 axon trn2 cores: 99.4-100us when the chip is unthrottled,
113-118us under the ~0.5-util SW power throttle that hits most runs
(baseline kernel: 114.5 min / 115-134 typical under the same conditions).
The stream is SDMA-fabric-bound: 33.6MB HBM read/core at ~26GB/s/engine
across 16 engines gives an ~81us engine-busy floor, plus ~7us runtime
prologue and ~7us drain/output tail.
"""

import numpy as np

N_CORES = 8
B, C, D, H, W = 2, 16, 64, 128, 128
HW = H * W            # 16384 free elems per (b, partition)
P = 128               # (C=16) x (D_local=8) partitions
D_LOCAL = D // N_CORES
MM_N = 512            # one PSUM bank of f32

# Per-b DMA segment plans: mids big for DMA/ACT efficiency, small tail so the
# last chunk's LN->DVE->PE chain after the final byte is short.
SEGS_B0 = (1024, 2048, 2048, 2048, 2048, 2048, 2048, 2048, 1024)
SEGS_B1 = (2048, 2048, 2048, 2048, 2048, 2048, 2048, 1024, 512, 512)


def build_bass_kernel(segs_b0=SEGS_B0, segs_b1=SEGS_B1,
                      pin_bufs=14, tin_bufs=3, uv_bufs=4, m_bufs=4,
                      sub=2048, mul_lag=1, head_scalar=2, tail_prefetch=True,
                      direct_reduce=True):
    """Build the per-core Bass/Tile kernel.

    Inputs  : pred, true [B, 128, free] f32 (shard, class*d_local on axis 1)
              wf [128, 1] bf16 (per-partition class weight)
    Outputs : out_m [1, 1] f32 = sum_p wf[p] * sum_e (t*(u-v) + v)[p, e]
    """
    import concourse.bacc as bacc
    import concourse.mybir as mybir
    import concourse.tile as tile

    f32 = mybir.dt.float32
    bf16 = mybir.dt.bfloat16
    AF = mybir.ActivationFunctionType

    segs_per_b = [list(segs_b0), list(segs_b1)]
    for segs in segs_per_b:
        assert sum(segs) == HW, segs
    plan = []                       # (b, offset, seg)
    total_mm = 0
    for b in range(B):
        off = 0
        for seg in segs_per_b[b]:
            plan.append((b, off, seg))
            total_mm += 2 * max(1, seg // MM_N)
            off += seg

    nc = bacc.Bacc("TRN2", target_bir_lowering=False, debug=False,
                   num_devices=N_CORES)
    pred_d = nc.dram_tensor("pred", [B, P, HW], f32, kind="ExternalInput")
    true_d = nc.dram_tensor("true", [B, P, HW], f32, kind="ExternalInput")
    wf_d = nc.dram_tensor("wf", [P, 1], bf16, kind="ExternalInput")
    outm_d = nc.dram_tensor("out_m", [1, 1], f32, kind="ExternalOutput")

    with tile.TileContext(nc) as tc:
        n_head = sum(1 for s in segs_b0 if s > sub)
        with (
            tc.tile_pool(name="headp", bufs=max(n_head, 1)) as headp,
            tc.tile_pool(name="pin", bufs=pin_bufs) as pin,
            tc.tile_pool(name="tin", bufs=tin_bufs) as tin,
            tc.tile_pool(name="uv", bufs=uv_bufs) as uvp,
            tc.tile_pool(name="mp", bufs=m_bufs) as mp,
            tc.tile_pool(name="small", bufs=1) as small,
            tc.tile_pool(name="psum", bufs=1, space="PSUM") as psump,
        ):
            wf_t = small.tile([P, 1], bf16, tag="wf")
            nc.gpsimd.dma_start(wf_t[:], wf_d[:])
            # prefetch the final true-chunk at stream start: the very last
            # DVE mul then never waits on a fresh DMA-completion semaphore
            # (~1.5us off the critical tail)
            last_i = len(plan) - 1
            tail_t = None
            if tail_prefetch:
                b_l, off_l, seg_l = plan[last_i]
                tail_t = small.tile([P, seg_l], bf16, tag="tail_t")
                nc.gpsimd.dma_start(tail_t[:],
                                    true_d[b_l, :, off_l:off_l + seg_l])
            acc = psump.tile([1, MM_N], f32, tag="acc")
            # warm up the Ln table set so the first real ACTIVATE doesn't pay
            # the ~2.7us ACT_TABLE_LOAD after its data lands. Input comes from
            # a memset (not the wf DMA) so the warm-up never blocks the ACT
            # FIFO behind a DMA-completion semaphore.
            warm_in = small.tile([P, 1], f32, tag="warm_in")
            nc.vector.memset(warm_in[:], 1.0)
            warm = small.tile([P, 1], bf16, tag="warm")
            nc.scalar.activation(warm[:], warm_in[:], AF.Ln, bias=1.0,
                                 scale=1.0)

            mm_i = 0
            # Pipeline the t-dependent DVE muls `mul_lag` sub-chunks behind
            # the subs: a mul waiting on its true-chunk DMA must not
            # head-of-line-block the next sub in DVE's FIFO (that stall
            # cascades: uv recycling -> ACT -> pin recycling -> pred DMA).
            pending = []        # (m_tile, t_tile, t_slice, width)

            def mm(src, w):
                nonlocal mm_i
                for q in range(max(1, w // MM_N)):
                    qq = slice(q * MM_N, min((q + 1) * MM_N, w))
                    nc.tensor.matmul(acc[:, 0:qq.stop - qq.start],
                                     wf_t[:], src[:, qq],
                                     start=(mm_i == 0),
                                     stop=(mm_i == total_mm - 1))
                    mm_i += 1

            def flush_one():
                m_t, t_t, tss, w = pending.pop(0)
                nc.vector.tensor_mul(m_t[:], t_t[:, tss], m_t[:])
                mm(m_t, w)

            for pi, (b, off, seg) in enumerate(plan):
                # ramp segs larger than `sub` live in their own pool so they
                # don't inflate every recycled pin slot
                p_pool = headp if (pi < n_head and seg > sub) else pin
                p_t = p_pool.tile([P, seg], f32, tag="p")
                sl = slice(off, off + seg)
                # a few early pred DMAs issue from the (still idle) scalar
                # sequencer's HWDGE ring in parallel with sync's, so the SDMA
                # queues fill at 2x rate during the ramp
                p_eng = nc.scalar if (0 < pi <= 2 * head_scalar
                                      and pi % 2) else nc.sync
                p_eng.dma_start(p_t[:], pred_d[b, :, sl])
                if pi == last_i and tail_t is not None:
                    t_t = tail_t
                else:
                    t_t = tin.tile([P, seg], bf16, tag="t")
                    # f32 -> bf16 cast inline (SWDGE-only feature)
                    nc.gpsimd.dma_start(t_t[:], true_d[b, :, sl])
                s_off = 0
                while s_off < seg:
                    s_sz = min(sub, seg - s_off)
                    ss = slice(s_off, s_off + s_sz)
                    u = uvp.tile([P, s_sz], bf16, tag="u")
                    v = uvp.tile([P, s_sz], bf16, tag="v")
                    nc.scalar.activation(u[:], p_t[:, ss], AF.Ln,
                                         bias=0.0, scale=1.0)
                    nc.scalar.activation(v[:], p_t[:, ss], AF.Ln,
                                         bias=1.0, scale=-1.0)
                    # acc += wf.T @ v (v is ready first; PE runs these while
                    # DVE forms m), later acc += wf.T @ m
                    mm(v, s_sz)
                    # d = u - v into a separate tile so u/v recycle without
                    # waiting on the t-gated mul
                    m_t = mp.tile([P, s_sz], bf16, tag="m")
                    nc.vector.tensor_sub(m_t[:], u[:], v[:])
                    pending.append((m_t, t_t, ss, s_sz))
                    while len(pending) > mul_lag:
                        flush_one()
                    s_off += s_sz
            while pending:
                flush_one()
            assert mm_i == total_mm

            outm_t = small.tile([1, 1], f32, tag="outm")
            if direct_reduce:
                nc.vector.reduce_sum(outm_t[:], acc[:],
                                     axis=mybir.AxisListType.X)
            else:
                accm_sb = small.tile([1, MM_N], f32, tag="accm_sb")
                nc.vector.tensor_copy(accm_sb[:], acc[:])
                nc.vector.reduce_sum(outm_t[:], accm_sb[:],
                                     axis=mybir.AxisListType.X)
            nc.sync.dma_start(outm_d[:], outm_t[:])

    nc.compile()
    return nc


_NC_CACHE = {}


def _get_nc():
    if "nc" not in _NC_CACHE:
        import json
        import os

        opts = json.loads(os.environ.get("KERNEL_OPTS", "{}"))
        for k in ("segs_b0", "segs_b1"):
            if k in opts:
                opts[k] = tuple(opts[k])
        _NC_CACHE["nc"] = build_bass_kernel(**opts)
    return _NC_CACHE["nc"]


def _bf16_round(x):
    """Round f32 array to bf16 values (kept in f32 representation)."""
    xi = np.asarray(x, dtype=np.float32).view(np.uint32)
    rounded = ((xi + 0x7FFF + ((xi >> 16) & 1)) & 0xFFFF0000).astype(np.uint32)
    return rounded.view(np.float32)


def shard_inputs(pred, true, weight):
    """Full [B,C,D,H,W] -> per-core in_maps."""
    import ml_dtypes

    wtile = np.repeat(np.asarray(weight, np.float32), D_LOCAL).reshape(P, 1)
    wf = wtile.astype(ml_dtypes.bfloat16)
    in_maps = []
    for i in range(N_CORES):
        d0 = i * D_LOCAL
        ps = np.ascontiguousarray(
            pred[:, :, d0:d0 + D_LOCAL].reshape(B, P, HW))
        ts = np.ascontiguousarray(
            true[:, :, d0:d0 + D_LOCAL].reshape(B, P, HW))
        in_maps.append({"pred": ps, "true": ts, "wf": wf})
    return in_maps


def combine(out_ms, weight):
    """out_ms [n_cores] scalars; weight [16] f32."""
    wt = _bf16_round(np.repeat(np.asarray(weight, np.float32), D_LOCAL))
    m = float(B * D * H * W)
    w_sum = wt.astype(np.float64)[::D_LOCAL].sum()   # sum of bf16 class weights
    total = float(np.asarray(out_ms, np.float64).sum())
    return np.float32(-total / (m * w_sum))


def kernel(pred, true, weight, _trace=False):
    from concourse.bass_utils import run_bass_kernel_spmd

    nc = _get_nc()
    in_maps = shard_inputs(np.asarray(pred), np.asarray(true), weight)
    res = run_bass_kernel_spmd(nc, in_maps, core_ids=list(range(N_CORES)),
                               trace=_trace)
    out_ms = [r["out_m"][0, 0] for r in res.results]
    out = combine(out_ms, weight)
    if _trace:
        return out, res
    return out
